# revision 13
# baseline (speedup 1.0000x reference)
"""Trainium2 Bass kernel for NeuralTensorLayer (order-1/2/3 polynomial layer).

    out[b,l] = bias[l] + sum_i X[b,i] W1[i,l]
             + sum_ij X[b,i] X[b,j] W2[i,j,l]
             + sum_ijk X[b,i] X[b,j] X[b,k] W3[i,j,k,l]

with B=32768, D=K=32, data-parallel over 8 NeuronCores (4096 rows each).

Strategy (per core):
  * (i,j) symmetry: 528 pairs i<=j against host-symmetrized weights, plus a
    single virtual "ones" contraction row (x_32 == 1) that carries W1 on the
    k-grid and bias on the out_low column -> 529 contraction rows in chunks
    of [128,128,128,128,17].
  * PSUM layout [128, 1056] f32: l-major k-grid col l*32+k in two bank-
    aligned 512-col halves, plus a contiguous 32-col out_low block (W2
    pairs + bias).  All matmul outputs are contiguous 2D APs (strided PSUM
    matmul writes pay a ~2-4x per-segment penalty on TRN2).
  * Pair operands arrive host-pregathered, packed [128, 5, B] bf16, fetched
    on two parallel DMA rings (XE via sync HWDGE, XR via gpsimd SWDGE) in
    two half-supertile pieces so the first Z build starts early; the DVE
    builds Z^T = XE*XR in two multiplies (2x bf16 mode).
  * Post per tile: ScalarE stages the k-grid to SBUF bf16 and casts out_low
    to f32; DVE multiplies by x (broadcast over l, 2x bf16), reduces over
    k=32, and adds out_low.  One batched output DMA per supertile.
  * Redundant LDWEIGHTS are stripped from the BIR before codegen (matmuls
    sharing a stationary operand across the three column-splits).
"""

import numpy as np
import ml_dtypes
from contextlib import ExitStack

import concourse.bass as bass
import concourse.bacc as bacc
import concourse.tile as tile
from concourse import mybir
from concourse import bass_utils

BF16 = ml_dtypes.bfloat16

B, D, KOUT = 32768, 32, 32
NCORES = 8
BLOC = B // NCORES          # 4096 rows per core
P = 128                     # rows per tile
SUPER = 4                   # tiles per supertile
SP = SUPER * P              # 512
NSUPER = BLOC // SP         # 8
NPAIRS = D * (D + 1) // 2   # 528
NROWS = NPAIRS + 1          # + ones row
NCHUNK = 5
CHUNK_P = [128, 128, 128, 128, 17]  # partitions per contraction chunk
NGRID = KOUT * D            # 1024 k-grid psum cols, col = l*32+k
NCOL = NGRID + KOUT         # + 32 out_low cols

PAIRS = [(i, j) for i in range(D) for j in range(i, D)]
I_P = np.array([p[0] for p in PAIRS], np.int32)
J_P = np.array([p[1] for p in PAIRS], np.int32)

F32 = mybir.dt.float32
BF = mybir.dt.bfloat16


# Drop redundant LDWEIGHTS from the BIR before walrus codegen: matmuls that
# share a stationary operand (the three column-splits per contraction chunk)
# each carry their own Ldweights (walrus's ldw-opt pass is disabled/broken).
# A load is elided when the previous PE weight-op in SCHEDULED order has a
# byte-identical weight AP and the load itself carries no semaphore
# waits/updates (so the PE weight registers provably still hold the same
# data and no sync edge is lost).
def _dedup_ldweights(bir_json: bytes) -> bytes:
    import json as _json

    d = _json.loads(bir_json)
    for fn in d.get("functions", []):
        for blk in fn.get("blocks", []):
            out = []
            last = None
            for i in blk.get("instructions", []):
                if i.get("engine") == "PE" and i.get("opcode") in ("Ldweights", "Matmult"):
                    w = i["ins"][-1] if i["opcode"] == "Matmult" else i["ins"][0]
                    key = (w.get("memref"), w.get("offset"), _json.dumps(w.get("ap")),
                           w.get("dtype"), _json.dumps(i.get("tile_position")),
                           _json.dumps(i.get("tile_size")), i.get("perf_mode"))
                    if i["opcode"] == "Ldweights":
                        si = i.get("sync_info") or {}
                        if (key == last and not si.get("on_wait")
                                and not si.get("on_update")):
                            continue
                        last = key
                    else:
                        last = key
                elif i.get("engine") == "PE":
                    last = None  # unknown PE op: invalidate weight-reuse state
                out.append(i)
            blk["instructions"] = out
    return _json.dumps(d).encode()


if not getattr(bass_utils, "_ldw_dedup_patched", False):
    _orig_compile_bir_kernel = bass_utils.compile_bir_kernel

    def _compile_bir_kernel_dedup(bir_json, tmpdir, neff_name="file.neff"):
        return _orig_compile_bir_kernel(_dedup_ldweights(bir_json), tmpdir, neff_name)

    bass_utils.compile_bir_kernel = _compile_bir_kernel_dedup
    import concourse.bass2jax as _b2j

    _b2j.compile_bir_kernel = _compile_bir_kernel_dedup
    bass_utils._ldw_dedup_patched = True


def _pack_weights(W1, W2, W3, bias):
    W1 = np.asarray(W1, np.float64)
    W2 = np.asarray(W2, np.float64)
    W3 = np.asarray(W3, np.float64)
    bias = np.asarray(bias, np.float64).reshape(KOUT)
    Wcat = np.zeros((NCHUNK, 128, NCOL), np.float64)
    for p, (i, j) in enumerate(PAIRS):
        c, pp = divmod(p, 128)
        if i < j:
            w3 = W3[i, j] + W3[j, i]   # [k, l]
            w2 = W2[i, j] + W2[j, i]   # [l]
        else:
            w3 = W3[i, i]
            w2 = W2[i, i]
        Wcat[c, pp, :NGRID] = w3.T.reshape(-1)     # col l*32+k
        Wcat[c, pp, NGRID:] = w2                   # out_low block
    c, pp = divmod(NPAIRS, 128)                    # ones row
    Wcat[c, pp, :NGRID] = W1.T.reshape(-1)         # col l*32+k = W1[k, l]
    Wcat[c, pp, NGRID:] = bias
    return Wcat.reshape(NCHUNK * 128, NCOL).astype(np.float32).astype(BF16)


def _build_module():
    nc = bacc.Bacc("TRN2", target_bir_lowering=False, debug=False,
                   enable_asserts=False)
    XBd = nc.dram_tensor("XB", [128, NSUPER * SUPER * D], BF,
                         kind="ExternalInput").ap()
    XEd = nc.dram_tensor("XE", [128, NCHUNK, BLOC], BF, kind="ExternalInput").ap()
    XRd = nc.dram_tensor("XR", [128, NCHUNK, BLOC], BF, kind="ExternalInput").ap()
    WCd = nc.dram_tensor("WCAT", [NCHUNK, 128, NCOL], BF,
                         kind="ExternalInput").ap()
    OUTd = nc.dram_tensor("OUT", [BLOC, KOUT], F32, kind="ExternalOutput").ap()

    with ExitStack() as ctx:
        tc = ctx.enter_context(tile.TileContext(nc))
        consts = ctx.enter_context(tc.tile_pool(name="consts", bufs=1))
        xepool = ctx.enter_context(tc.tile_pool(name="xepool", bufs=3))
        zpool = ctx.enter_context(tc.tile_pool(name="zpool", bufs=2))
        spool = ctx.enter_context(tc.tile_pool(name="spool", bufs=3))
        upool = ctx.enter_context(tc.tile_pool(name="upool", bufs=3))
        rpool = ctx.enter_context(tc.tile_pool(name="rpool", bufs=3))
        opool = ctx.enter_context(tc.tile_pool(name="opool", bufs=2))
        t3ps = ctx.enter_context(tc.tile_pool(name="t3ps", bufs=2, space="PSUM"))

        # chunk-0 weights first: the opening matmuls need them earliest
        w_sb = [None] * NCHUNK
        for c in (0, 1):
            w = consts.tile([128, NCOL], BF, tag=f"w_{c}")
            nc.scalar.dma_start(out=w, in_=WCd[c])
            w_sb[c] = w
        xball = consts.tile([128, NSUPER * SUPER * D], BF, tag="xball")
        nc.scalar.dma_start(out=xball, in_=XBd)
        for c in (2, 3, 4):
            w = consts.tile([128, NCOL], BF, tag=f"w_{c}")
            nc.scalar.dma_start(out=w, in_=WCd[c])
            w_sb[c] = w

        def fetch(s):
            xe = xepool.tile([128, NCHUNK * SP], BF, tag="xe")
            nc.sync.dma_start(out=xe, in_=XEd[:, :, s * SP:(s + 1) * SP])
            xr = xepool.tile([128, NCHUNK * SP], BF, tag="xr")
            nc.gpsimd.dma_start(out=xr, in_=XRd[:, :, s * SP:(s + 1) * SP])
            return xe, xr

        # supertile 0 arrives in two halves so the first matmuls start as
        # soon as ~0.6 MB (not 2.6 MB) has landed; the startup is HBM-
        # bandwidth-bound behind a ~7us fixed preamble.
        HSP = SP // 2
        z0 = []
        for h in range(2):
            xh = xepool.tile([128, NCHUNK * HSP], BF, tag=f"xe0_{h}")
            nc.sync.dma_start(out=xh, in_=XEd[:, :, h * HSP:(h + 1) * HSP])
            rh = xepool.tile([128, NCHUNK * HSP], BF, tag=f"xr0_{h}")
            nc.gpsimd.dma_start(out=rh, in_=XRd[:, :, h * HSP:(h + 1) * HSP])
            zh = zpool.tile([128, NCHUNK * HSP], BF, tag=f"z0_{h}")
            nc.vector.tensor_mul(zh, xh, rh)
            z0.append(zh)
        fetched = [None, fetch(1)]
        for s in range(NSUPER):
            if s > 0:
                xe, xr = fetched[s % 2]
                z = zpool.tile([128, NCHUNK * SP], BF, tag="z")
                nc.vector.tensor_mul(z, xe, xr)
            if s + 2 < NSUPER:
                fetched[s % 2] = fetch(s + 2)
            obuf = opool.tile([128, SUPER * KOUT], F32, tag="obuf")
            for t in range(SUPER):
                t3 = t3ps.tile([P, NCOL], F32, tag="t3")
                for c in range(NCHUNK):
                    pcp = CHUNK_P[c]
                    first, last = c == 0, c == NCHUNK - 1
                    if s == 0:
                        zc = z0[t // 2][:pcp, c * HSP + (t % 2) * P:
                                        c * HSP + (t % 2 + 1) * P]
                    else:
                        zc = z[:pcp, c * SP + t * P: c * SP + (t + 1) * P]
                    for n0, n1 in ((0, 512), (512, 1024), (1024, NCOL)):
                        nc.tensor.matmul(t3[:, n0:n1], zc,
                                         w_sb[c][:pcp, n0:n1],
                                         start=first, stop=last)
                staged = spool.tile([P, NGRID], BF, tag="staged")
                nc.scalar.copy(out=staged, in_=t3[:, :NGRID])
                olf = rpool.tile([P, KOUT], F32, tag="olf")
                nc.scalar.copy(out=olf, in_=t3[:, NGRID:NCOL])
                u = upool.tile([P, NGRID], BF, tag="u")
                off = (s * SUPER + t) * D
                xk = xball[:, off:off + D].unsqueeze(1).broadcast_to(
                    [P, KOUT, D])
                nc.vector.tensor_mul(
                    u[:, :].rearrange("p (l k) -> p l k", k=D),
                    staged[:, :].rearrange("p (l k) -> p l k", k=D),
                    xk,
                )
                rtmp = rpool.tile([P, KOUT], F32, tag="rtmp")
                nc.vector.reduce_sum(
                    out=rtmp,
                    in_=u[:, :].rearrange("p (l k) -> p l k", k=D),
                    axis=mybir.AxisListType.X,
                )
                nc.vector.tensor_add(obuf[:, t * KOUT:(t + 1) * KOUT],
                                     rtmp, olf)
            nc.scalar.dma_start(
                out=OUTd[s * SP:(s + 1) * SP, :].rearrange(
                    "(t p) l -> p t l", t=SUPER),
                in_=obuf[:, :].rearrange("p (t l) -> p t l", l=KOUT),
            )
    nc.compile()
    return nc


_CACHE = {}


def _get_module():
    if "nc" not in _CACHE:
        _CACHE["nc"] = _build_module()
    return _CACHE["nc"]


def kernel(X, W1, W2, W3, bias):
    X = np.ascontiguousarray(np.asarray(X, np.float32))
    Wcat = _pack_weights(W1, W2, W3, bias).reshape(NCHUNK, 128, NCOL)

    nc = _get_module()
    Xb = X.astype(BF16)                      # [B, D] bf16 (single rounding point)
    XbT = np.ascontiguousarray(Xb.T)         # [D, B] bf16
    npad = NCHUNK * 128 - NROWS
    ones_row = np.ones((1, B), BF16)
    zpad = np.zeros((npad, B), BF16)
    XE = np.concatenate([XbT[I_P], ones_row, zpad], 0).reshape(NCHUNK, 128, B)
    XR = np.concatenate([XbT[J_P], ones_row, zpad], 0).reshape(NCHUNK, 128, B)
    # packed layouts: [core][part 128][chunk 5][bloc]
    XEp = XE.reshape(NCHUNK, 128, NCORES, BLOC).transpose(2, 1, 0, 3)
    XRp = XR.reshape(NCHUNK, 128, NCORES, BLOC).transpose(2, 1, 0, 3)
    # [core][part 128][supertile*tile][d]
    XBp = Xb.reshape(NCORES, NSUPER, SUPER, P, D).transpose(
        0, 3, 1, 2, 4).reshape(NCORES, P, NSUPER * SUPER * D)
    in_maps = [
        {
            "XB": np.ascontiguousarray(XBp[c]),
            "XE": np.ascontiguousarray(XEp[c]),
            "XR": np.ascontiguousarray(XRp[c]),
            "WCAT": Wcat,
        }
        for c in range(NCORES)
    ]
    res = bass_utils.run_bass_kernel_spmd(nc, in_maps, core_ids=list(range(NCORES)))
    _CACHE["last_results"] = res
    out = np.concatenate([np.asarray(res.results[c]["OUT"]) for c in range(NCORES)], 0)
    return out.astype(np.float32)
